# revision 3
# baseline (speedup 1.0000x reference)
"""Trainium2 Bass kernel for nn_AttentionBlock (B=4, T=2048, C=K=V=1024).

Self-contained: builds one SPMD Bass/Tile program, runs it on 8 NeuronCores
via run_bass_kernel_spmd, and reassembles the full output on the host.

Math (matches the reference):
  q/k/v = x @ W + b ; logits[b,t,s] = q.k, causal mask s<=t ;
  probs = softmax(logits/sqrt(K), axis=t)   # over the QUERY axis
  read = probs @ v ; out = concat(x, read, axis=2)

Sharding (zero-collective): core = 2*b + h owns batch b and the interleaved
key/value tiles sigma = 2*i + h (interleaving balances the causal triangle).
Because the softmax normalizes over the query axis t and each core has ALL
queries for its own key columns, the softmax is fully core-local. Each core
computes q in full, k/v only for its own columns, exp-normalized scores
et[s_own, t], and an additive partial read_h[t, v] = et^T @ v_own. The host
sums the two partials per batch (the unshard step for this additive
tensor-parallel split) and concatenates x.

Numerics: phases A/B/C (q/k projections + scores) run fp8e4m3 with DoubleRow
perf mode (2 contraction subtiles per matmul, ~1.4x PE throughput); phases
D/E (v projection + probs@v) stay bf16 because the query-axis softmax gives
read rows with weight sums >> 1, amplifying v/probs quantization error
(end-to-end sim: D or E in fp8 -> 2.3e-2/2.9e-2 rel err vs the 2e-2 gate;
A/B/C in fp8 -> ~7e-3 total). PSUM accumulation is f32 throughout; exp and
the softmax normalizer stay f32. Output partials are bf16 (host sums f32).

Per-core phases (single dense PE pipeline, no collectives):
  A. qt [k, 2048] = wq.T @ x^T + bq          (fp8 DoubleRow, x^T streamed)
  B. kt_own [k, 1024] = wk.T @ xst + bk      (fp8 DoubleRow, xst resident)
  C. et_i = exp((kt_i.T @ qt + mask)/32) with fused row-sum (ACT
     accum_out) -> dinv = 1/rowsum; et_i *= dinv on DVE (fp8 DoubleRow MMs)
  D. v_own [1024, v] = xst.T @ wv + bv       (bf16)
  E. read_partial[t, v] = sum_i et_i.T @ v_own_i -> DRAM bf16
"""

from contextlib import ExitStack

import numpy as np

import concourse.mybir as mybir
import concourse.tile as tile
from concourse import bacc
from concourse._compat import with_exitstack

P = 128
B = 4
T = 2048
C = 1024
KD = 1024
VD = 1024
NCO = C // P
NKO = KD // P
NI = 8
SOWN = NI * P
SQRT_K = 32.0
F32 = mybir.dt.float32
CD = mybir.dt.bfloat16
F8 = mybir.dt.float8e4
DR = mybir.MatmulPerfMode.DoubleRow
SB = 512


@with_exitstack
def attn_body(ctx: ExitStack, tc, io):
    nc = tc.nc
    xt = io["xt"].ap().rearrange("(co ci) t -> ci co t", ci=P)
    xst8 = io["xst8"].ap().rearrange("(co ci) t -> ci co t", ci=P)
    xstb = io["xstb"].ap().rearrange("(co ci) t -> ci co t", ci=P)
    wk = io["wk"].ap().rearrange("(co ci) k -> ci co k", ci=P)
    wq = io["wq"].ap().rearrange("(co ci) k -> ci co k", ci=P)
    wv = io["wv"].ap().rearrange("(co ci) k -> ci co k", ci=P)

    const = ctx.enter_context(tc.tile_pool(name="const", bufs=1))
    bk_sb = const.tile([P, NKO], F32)
    bq_sb = const.tile([P, NKO], F32)
    mask_sb = const.tile([P, 2 * P], F32)
    nc.gpsimd.dma_start(bk_sb[:], io["bk2"].ap())
    nc.gpsimd.dma_start(bq_sb[:], io["bq2"].ap())
    nc.gpsimd.dma_start(mask_sb[:], io["maskbias"].ap())

    psum = ctx.enter_context(tc.tile_pool(name="psum", bufs=8, space="PSUM"))

    wp = ctx.enter_context(tc.tile_pool(name="wp", bufs=1, side="right"))
    wq_sb = wp.tile([P, NCO, KD], F8, name="wq_sb")
    wk_sb = wp.tile([P, NCO, KD], F8, name="wk_sb")
    wv_sb = wp.tile([P, NCO, VD], CD, name="wv_sb")
    bv_sb = wp.tile([P, VD], F32, name="bv_sb")
    xstp = ctx.enter_context(tc.tile_pool(name="xstp", bufs=1, side="right"))
    xst8_sb = xstp.tile([P, NCO, SOWN], F8)
    xstb_sb = xstp.tile([P, NCO, SOWN], CD)

    # ---------------- phase A: qt = wq.T @ xt + bq (full t) --------------
    # ko -> ci-pair -> sb order: the 4 sb-block matmuls of one (ko, ci-pair)
    # share a stationary DoubleRow weight tile [P, 2, P].
    qtp = ctx.enter_context(tc.tile_pool(name="qtp", bufs=1))
    qt = qtp.tile([P, NKO, T], F8, tag="qt")
    xtp_cm = tc.tile_pool(name="xtp", bufs=1, side="right")
    xtp = xtp_cm.__enter__()
    xt_sb = xtp.tile([P, NCO, T], F8)
    for ci in range(NCO):
        nc.sync.dma_start(wq_sb[:, ci, :], wq[:, ci, :])
        nc.sync.dma_start(xt_sb[:, ci, :], xt[:, ci, :])
    NBLK = T // SB
    for ko in range(NKO):
        pss = [
            psum.tile([P, SB], F32, tag="ps", name=f"psA{ko}_{sb}")
            for sb in range(NBLK)
        ]
        for ci in range(0, NCO, 2):
            for sb in range(NBLK):
                nc.tensor.matmul(
                    pss[sb][:],
                    wq_sb[:, ci : ci + 2, ko * P : (ko + 1) * P],
                    xt_sb[:, ci : ci + 2, sb * SB : (sb + 1) * SB],
                    start=(ci == 0),
                    stop=(ci == NCO - 2),
                    perf_mode=DR,
                )
        for sb in range(NBLK):
            nc.vector.tensor_add(
                qt[:, ko, sb * SB : (sb + 1) * SB],
                pss[sb][:],
                bq_sb[:, ko : ko + 1].to_broadcast((P, SB)),
            )
    xtp_cm.__exit__(None, None, None)

    # loads for phases B and D, queued behind phase A's streams
    for sb in range(SOWN // SB):
        nc.sync.dma_start(
            xst8_sb[:, :, sb * SB : (sb + 1) * SB],
            xst8[:, :, sb * SB : (sb + 1) * SB],
        )
    for sb in range(SOWN // SB):
        nc.sync.dma_start(
            xstb_sb[:, :, sb * SB : (sb + 1) * SB],
            xstb[:, :, sb * SB : (sb + 1) * SB],
        )
    for ko in range(NKO):
        nc.sync.dma_start(
            wk_sb[:, :, ko * P : (ko + 1) * P], wk[:, :, ko * P : (ko + 1) * P]
        )
    nc.sync.dma_start(wv_sb[:], wv)
    nc.sync.dma_start(bv_sb[:], io["bv2"].ap())

    # ---------------- phase B: kt_own = wk.T @ xst + bk ----------------
    ktp = ctx.enter_context(tc.tile_pool(name="ktp", bufs=1))
    kt = ktp.tile([P, NKO, SOWN], F8, tag="kt")
    for ko in range(NKO):
        pss = [
            psum.tile([P, SB], F32, tag="ps", name=f"psB{ko}_{sb}")
            for sb in range(SOWN // SB)
        ]
        for ci in range(0, NCO, 2):
            for sb in range(SOWN // SB):
                nc.tensor.matmul(
                    pss[sb][:],
                    wk_sb[:, ci : ci + 2, ko * P : (ko + 1) * P],
                    xst8_sb[:, ci : ci + 2, sb * SB : (sb + 1) * SB],
                    start=(ci == 0),
                    stop=(ci == NCO - 2),
                    perf_mode=DR,
                )
        for sb in range(SOWN // SB):
            nc.vector.tensor_add(
                kt[:, ko, sb * SB : (sb + 1) * SB],
                pss[sb][:],
                bk_sb[:, ko : ko + 1].to_broadcast((P, SB)),
            )

    # ------ phase C: et_i = exp((kt_i.T @ qt + mask)/32); scale ------
    etp = ctx.enter_context(tc.tile_pool(name="etp", bufs=1, side="right"))
    et = etp.tile([P, NI, T], CD, tag="et")
    dsum = const.tile([P, NI], F32, name="dsum")
    dinv = const.tile([P, NI], F32, name="dinv")
    dparts = const.tile([P, NI, 4], F32, name="dparts")
    for i in range(NI):
        tstart = 2 * i * P
        nchunk = 0
        t0 = tstart
        while t0 < T:
            w = min(SB, T - t0)
            ps = psum.tile([P, SB], F32, tag="ps")
            for ko in range(0, NKO, 2):
                nc.tensor.matmul(
                    ps[:, :w],
                    kt[:, ko : ko + 2, i * P : (i + 1) * P],
                    qt[:, ko : ko + 2, t0 : t0 + w],
                    start=(ko == 0),
                    stop=(ko == NKO - 2),
                    perf_mode=DR,
                )
            if nchunk == 0:
                nc.vector.tensor_add(ps[:, : 2 * P], ps[:, : 2 * P], mask_sb[:])
            nc.scalar.activation(
                et[:, i, t0 : t0 + w],
                ps[:, :w],
                mybir.ActivationFunctionType.Exp,
                scale=1.0 / SQRT_K,
                accum_out=dparts[:, i, nchunk : nchunk + 1],
            )
            t0 += w
            nchunk += 1
        nc.vector.tensor_copy(dsum[:, i : i + 1], dparts[:, i, 0:1])
        for c in range(1, nchunk):
            nc.vector.tensor_add(
                dsum[:, i : i + 1], dsum[:, i : i + 1], dparts[:, i, c : c + 1]
            )
        nc.vector.reciprocal(dinv[:, i : i + 1], dsum[:, i : i + 1])
        nc.vector.tensor_mul(
            et[:, i, tstart:],
            et[:, i, tstart:],
            dinv[:, i : i + 1].to_broadcast((P, T - tstart)),
        )

    # ---------------- phase D: v_own = xst.T @ wv + bv ----------------
    vop = ctx.enter_context(tc.tile_pool(name="vop", bufs=1))
    v_own = vop.tile([P, NI, VD], CD)
    for jl in range(NI):
        pss = [
            psum.tile([P, SB], F32, tag="ps", name=f"psD{jl}_{vb}")
            for vb in range(VD // SB)
        ]
        for ci in range(NCO):
            for vb in range(VD // SB):
                nc.tensor.matmul(
                    pss[vb][:],
                    xstb_sb[:, ci, jl * P : (jl + 1) * P],
                    wv_sb[:, ci, vb * SB : (vb + 1) * SB],
                    start=(ci == 0),
                    stop=(ci == NCO - 1),
                )
        for vb in range(VD // SB):
            nc.vector.tensor_add(
                v_own[:, jl, vb * SB : (vb + 1) * SB],
                pss[vb][:],
                bv_sb[:, vb * SB : (vb + 1) * SB],
            )

    # ------------- phase E: read_partial = sum_i et_i.T @ v_i -------------
    read_out = io["read_out"].ap()
    with tc.tile_pool(name="rout", bufs=8) as rout:
        for g in range(T // P):
            ni = g // 2 + 1
            pss = [
                psum.tile([P, SB], F32, tag="ps", name=f"psE{g}_{vb}")
                for vb in range(VD // SB)
            ]
            for i in range(ni):
                for vb in range(VD // SB):
                    nc.tensor.matmul(
                        pss[vb][:],
                        et[:, i, g * P : (g + 1) * P],
                        v_own[:, i, vb * SB : (vb + 1) * SB],
                        start=(i == 0),
                        stop=(i == ni - 1),
                    )
            for vb in range(VD // SB):
                ro = rout.tile([P, SB], CD, tag="rout")
                if (2 * g + vb) % 2 == 0:
                    nc.scalar.copy(ro[:], pss[vb][:])
                    nc.sync.dma_start(
                        read_out[g * P : (g + 1) * P, vb * SB : (vb + 1) * SB],
                        ro[:],
                    )
                else:
                    nc.vector.tensor_copy(ro[:], pss[vb][:])
                    nc.gpsimd.dma_start(
                        read_out[g * P : (g + 1) * P, vb * SB : (vb + 1) * SB],
                        ro[:],
                    )


def _build_nc(num_devices=8):
    nc = bacc.Bacc(
        "TRN2", target_bir_lowering=False, debug=False, num_devices=num_devices
    )
    io = {}
    io["xt"] = nc.dram_tensor("xt", [C, T], F8, kind="ExternalInput")
    io["xst8"] = nc.dram_tensor("xst8", [C, SOWN], F8, kind="ExternalInput")
    io["xstb"] = nc.dram_tensor("xstb", [C, SOWN], CD, kind="ExternalInput")
    io["wq"] = nc.dram_tensor("wq", [C, KD], F8, kind="ExternalInput")
    io["wk"] = nc.dram_tensor("wk", [C, KD], F8, kind="ExternalInput")
    io["wv"] = nc.dram_tensor("wv", [C, KD], CD, kind="ExternalInput")
    io["bk2"] = nc.dram_tensor("bk2", [P, NKO], F32, kind="ExternalInput")
    io["bq2"] = nc.dram_tensor("bq2", [P, NKO], F32, kind="ExternalInput")
    io["bv2"] = nc.dram_tensor("bv2", [P, VD], F32, kind="ExternalInput")
    io["maskbias"] = nc.dram_tensor(
        "maskbias", [P, 2 * P], F32, kind="ExternalInput"
    )
    io["read_out"] = nc.dram_tensor(
        "read_out", [T, VD], CD, kind="ExternalOutput"
    )
    with tile.TileContext(nc) as tc:
        attn_body(tc, io)
    nc.compile()
    return nc


def _own_cols(h):
    idx = []
    for i in range(NI):
        g = 2 * i + h
        idx.extend(range(g * P, (g + 1) * P))
    return np.array(idx)


def _make_in_maps(x, Wq, bq, Wk, bk, Wv, bv):
    import ml_dtypes

    bf16 = ml_dtypes.bfloat16
    f8 = ml_dtypes.float8_e4m3
    x = np.asarray(x, np.float32)
    Wq8, Wk8 = (
        np.ascontiguousarray(np.asarray(w, np.float32).astype(f8))
        for w in (Wq, Wk)
    )
    Wvb = np.ascontiguousarray(np.asarray(Wv, np.float32).astype(bf16))
    bq, bk, bv = (np.asarray(v, np.float32) for v in (bq, bk, bv))

    sr = np.arange(P)[:, None]
    tcc = np.arange(P)[None, :]
    tri = np.where(tcc >= sr, 0.0, -1e9).astype(np.float32)
    masks = {}
    for h in (0, 1):
        m = np.zeros((P, 2 * P), np.float32)
        if h == 0:
            m[:, :P] = tri
        else:
            m[:, :P] = -1e9
            m[:, P:] = tri
        masks[h] = m

    bk2 = np.ascontiguousarray(bk.reshape(NKO, P).T)
    bq2 = np.ascontiguousarray(bq.reshape(NKO, P).T)
    bv2 = np.ascontiguousarray(np.broadcast_to(bv[None, :], (P, VD)))

    in_maps = []
    for core in range(8):
        b, h = core // 2, core % 2
        xt_b = np.ascontiguousarray(x[b].T)
        own = _own_cols(h)
        in_maps.append(
            {
                "xt": np.ascontiguousarray(xt_b.astype(f8)),
                "xst8": np.ascontiguousarray(xt_b[:, own].astype(f8)),
                "xstb": np.ascontiguousarray(xt_b[:, own].astype(bf16)),
                "wq": Wq8,
                "wk": Wk8,
                "wv": Wvb,
                "bk2": bk2,
                "bq2": bq2,
                "bv2": bv2,
                "maskbias": masks[h],
            }
        )
    return in_maps


def _assemble_output(x, results):
    x = np.asarray(x, np.float32)
    out = np.empty((x.shape[0], T, C + VD), np.float32)
    out[:, :, :C] = x
    for b in range(x.shape[0]):
        out[b, :, C:] = results[2 * b]["read_out"].astype(np.float32) + results[
            2 * b + 1
        ]["read_out"].astype(np.float32)
    return out


_NC_CACHE = None


def _build():
    global _NC_CACHE
    if _NC_CACHE is None:
        _NC_CACHE = _build_nc(num_devices=8)
    return _NC_CACHE


def kernel(x, Wq, bq, Wk, bk, Wv, bv):
    from concourse.bass_utils import run_bass_kernel_spmd

    nc = _build()
    in_maps = _make_in_maps(x, Wq, bq, Wk, bk, Wv, bv)
    res = run_bass_kernel_spmd(nc, in_maps, core_ids=list(range(8)))
    return _assemble_output(x, res.results)


# revision 6
# speedup vs baseline: 1.9408x; 1.9408x over previous
"""Trainium2 Bass kernel for nn_AttentionBlock (B=4, T=2048, C=K=V=1024).

Self-contained: builds one SPMD Bass/Tile program, runs it on 8 NeuronCores
via run_bass_kernel_spmd, and reassembles the full output on the host.

Math (matches the reference):
  q/k/v = x @ W + b ; logits[b,t,s] = q.k, causal mask s<=t ;
  probs = softmax(logits/sqrt(K), axis=t)   # over the QUERY axis
  read = probs @ v ; out = concat(x, read, axis=2)

Sharding (zero-collective): core = 2*b + h owns batch b and the interleaved
key/value tiles sigma = 2*i + h (interleaving balances the causal triangle).
Because the softmax normalizes over the query axis t and each core has ALL
queries for its own key columns, the softmax is fully core-local. Each core
computes q in full, k/v only for its own columns, exp-normalized scores
et[s_own, t], and an additive partial read_h[t, v] = et^T @ v_own. The host
sums the two partials per batch and concatenates x.

Numerics: fp8e4m3 with DoubleRow perf mode (2 contraction subtiles per
matmul, 2x PE streaming throughput) everywhere EXCEPT the last two own-key
tiles (s >= ~1536) of phases D/E, which stay bf16: the query-axis softmax
makes read rows weight-sums >> 1 there, amplifying v/probs quantization
(measured: full-D/E fp8 -> 2.3e-2 rel err vs the 2e-2 gate; this split ->
1.11e-2 on the real inputs). The fp8 D/E tiles use a x16//16 scale split
(probs*16, v/16) so small probabilities clear fp8's 2^-9 subnormal floor.
PSUM accumulation is f32 throughout; exp and the softmax normalizer are f32.
Output partials are bf16 (host sums in f32).

Per-core phases (single dense PE pipeline, no collectives):
  A. qt [k, 2048] = wq.T @ x^T + bq          (fp8 DR, x^T streamed)
  B. kt_own [k, 1024] = wk.T @ xst + bk      (fp8 DR, xst resident)
  C. et_i = exp((kt_i.T @ qt + mask)/32) with fused row-sum (ACT
     accum_out); tiles 0-5 -> et8 = et*16/rowsum (DVE), 6-7 bf16 in place
  D. v_own = xst.T @ wv + bv   (tiles 0-5 fp8 DR -> v8=(v)/16, 6-7 bf16)
  E. read_partial[t, v] = sum_i et_i.T @ v_own_i -> DRAM bf16
     (fp8-DR tile pairs + bf16 tail; (16p).(v/16) = p.v exactly)
"""

from contextlib import ExitStack

import numpy as np

import concourse.mybir as mybir
import concourse.tile as tile
from concourse import bacc
from concourse._compat import with_exitstack

P = 128
B = 4
T = 2048
C = 1024
KD = 1024
VD = 1024
NCO = C // P
NKO = KD // P
NI = 8
NI8 = 6  # own-key tiles 0..NI8-1 run fp8 in phases D/E
SOWN = NI * P
SQRT_K = 32.0
ESC = 16.0  # fp8 scale split: et8 = p*ESC, v8 = v/ESC
F32 = mybir.dt.float32
CD = mybir.dt.bfloat16
F8 = mybir.dt.float8e4
DR = mybir.MatmulPerfMode.DoubleRow
ALU = mybir.AluOpType
SB = 512


@with_exitstack
def attn_body(ctx: ExitStack, tc, io):
    nc = tc.nc
    xt = io["xt"].ap().rearrange("(co ci) t -> ci co t", ci=P)
    xst8 = io["xst8"].ap().rearrange("(co ci) t -> ci co t", ci=P)
    xstb = io["xstb"].ap().rearrange("(co ci) t -> ci co t", ci=P)
    wk = io["wk"].ap().rearrange("(co ci) k -> ci co k", ci=P)
    wq = io["wq"].ap().rearrange("(co ci) k -> ci co k", ci=P)
    wv = io["wv"].ap().rearrange("(co ci) k -> ci co k", ci=P)
    wv8 = io["wv8"].ap().rearrange("(co ci) k -> ci co k", ci=P)

    const = ctx.enter_context(tc.tile_pool(name="const", bufs=1))
    bk_sb = const.tile([P, NKO], F32)
    bq_sb = const.tile([P, NKO], F32)
    mask_sb = const.tile([P, 2 * P], F32)
    nc.gpsimd.dma_start(bk_sb[:], io["bk2"].ap())
    nc.gpsimd.dma_start(bq_sb[:], io["bq2"].ap())
    nc.gpsimd.dma_start(mask_sb[:], io["maskbias"].ap())

    psum = ctx.enter_context(tc.tile_pool(name="psum", bufs=8, space="PSUM"))

    wp = ctx.enter_context(tc.tile_pool(name="wp", bufs=1, side="right"))
    wq_sb = wp.tile([P, NCO, KD], F8, name="wq_sb")
    wk_sb = wp.tile([P, NCO, KD], F8, name="wk_sb")
    wv_sb = wp.tile([P, NCO, VD], CD, name="wv_sb")
    wv8_sb = wp.tile([P, NCO, VD], F8, name="wv8_sb")
    bv_sb = wp.tile([P, VD], F32, name="bv_sb")
    bv16_sb = wp.tile([P, VD], F32, name="bv16_sb")
    xstp = ctx.enter_context(tc.tile_pool(name="xstp", bufs=1, side="right"))
    xst8_sb = xstp.tile([P, NCO, SOWN], F8)
    xstb_sb = xstp.tile([P, NCO, SOWN], CD)

    # ---------------- phase A: qt = wq.T @ xt + bq (full t) --------------
    qtp = ctx.enter_context(tc.tile_pool(name="qtp", bufs=1))
    qt = qtp.tile([P, NKO, T], F8, tag="qt")
    xtp_cm = tc.tile_pool(name="xtp", bufs=1, side="right")
    xtp = xtp_cm.__enter__()
    xt_sb = xtp.tile([P, NCO, T], F8)
    # load order tuned so the first (ko=0, ci-pair 0) matmuls unblock ASAP:
    # wq ci 0-1, then xt ci 0-1 in t-quarters, then the rest interleaved
    TQ = T // 4
    for ci in (0, 1):
        nc.sync.dma_start(wq_sb[:, ci, :], wq[:, ci, :])
    for tq in range(4):
        for ci in (0, 1):
            nc.sync.dma_start(
                xt_sb[:, ci, tq * TQ : (tq + 1) * TQ],
                xt[:, ci, tq * TQ : (tq + 1) * TQ],
            )
    for ci in range(2, NCO):
        nc.sync.dma_start(wq_sb[:, ci, :], wq[:, ci, :])
        nc.sync.dma_start(xt_sb[:, ci, :], xt[:, ci, :])
    NBLK = T // SB
    for ko in range(NKO):
        pss = [
            psum.tile([P, SB], F32, tag="ps", name=f"psA{ko}_{sb}")
            for sb in range(NBLK)
        ]
        for ci in range(0, NCO, 2):
            for sb in range(NBLK):
                nc.tensor.matmul(
                    pss[sb][:],
                    wq_sb[:, ci : ci + 2, ko * P : (ko + 1) * P],
                    xt_sb[:, ci : ci + 2, sb * SB : (sb + 1) * SB],
                    start=(ci == 0),
                    stop=(ci == NCO - 2),
                    perf_mode=DR,
                )
        for sb in range(NBLK):
            nc.vector.tensor_add(
                qt[:, ko, sb * SB : (sb + 1) * SB],
                pss[sb][:],
                bq_sb[:, ko : ko + 1].to_broadcast((P, SB)),
            )
    xtp_cm.__exit__(None, None, None)

    # loads for phases B and D, queued behind phase A's streams
    for sb in range(SOWN // SB):
        nc.sync.dma_start(
            xst8_sb[:, :, sb * SB : (sb + 1) * SB],
            xst8[:, :, sb * SB : (sb + 1) * SB],
        )
    for sb in range(SOWN // SB):
        nc.sync.dma_start(
            xstb_sb[:, :, sb * SB : (sb + 1) * SB],
            xstb[:, :, sb * SB : (sb + 1) * SB],
        )
    for ko in range(NKO):
        nc.sync.dma_start(
            wk_sb[:, :, ko * P : (ko + 1) * P], wk[:, :, ko * P : (ko + 1) * P]
        )
    nc.sync.dma_start(wv_sb[:], wv)
    nc.sync.dma_start(wv8_sb[:], wv8)
    nc.sync.dma_start(bv_sb[:], io["bv2"].ap())
    nc.sync.dma_start(bv16_sb[:], io["bv216"].ap())

    # ---------------- phase B: kt_own = wk.T @ xst + bk ----------------
    ktp = ctx.enter_context(tc.tile_pool(name="ktp", bufs=1))
    kt = ktp.tile([P, NKO, SOWN], F8, tag="kt")
    for ko in range(NKO):
        pss = [
            psum.tile([P, SB], F32, tag="ps", name=f"psB{ko}_{sb}")
            for sb in range(SOWN // SB)
        ]
        for ci in range(0, NCO, 2):
            for sb in range(SOWN // SB):
                nc.tensor.matmul(
                    pss[sb][:],
                    wk_sb[:, ci : ci + 2, ko * P : (ko + 1) * P],
                    xst8_sb[:, ci : ci + 2, sb * SB : (sb + 1) * SB],
                    start=(ci == 0),
                    stop=(ci == NCO - 2),
                    perf_mode=DR,
                )
        for sb in range(SOWN // SB):
            nc.vector.tensor_add(
                kt[:, ko, sb * SB : (sb + 1) * SB],
                pss[sb][:],
                bk_sb[:, ko : ko + 1].to_broadcast((P, SB)),
            )

    # ------ phase C: et_i = exp((kt_i.T @ qt + mask)/32); scale ------
    etp = ctx.enter_context(tc.tile_pool(name="etp", bufs=1, side="right"))
    et = etp.tile([P, NI, T], CD, tag="et")
    et8 = etp.tile([P, NI8, T], F8, tag="et8")
    dsum = const.tile([P, NI], F32, name="dsum")
    dinv = const.tile([P, NI], F32, name="dinv")
    dparts = const.tile([P, NI, 4], F32, name="dparts")
    for i in range(NI):
        tstart = 2 * i * P
        nchunk = 0
        t0 = tstart
        while t0 < T:
            w = min(SB, T - t0)
            ps = psum.tile([P, SB], F32, tag="ps")
            for ko in range(0, NKO, 2):
                nc.tensor.matmul(
                    ps[:, :w],
                    kt[:, ko : ko + 2, i * P : (i + 1) * P],
                    qt[:, ko : ko + 2, t0 : t0 + w],
                    start=(ko == 0),
                    stop=(ko == NKO - 2),
                    perf_mode=DR,
                )
            if nchunk == 0:
                nc.vector.tensor_add(ps[:, : 2 * P], ps[:, : 2 * P], mask_sb[:])
            nc.scalar.activation(
                et[:, i, t0 : t0 + w],
                ps[:, :w],
                mybir.ActivationFunctionType.Exp,
                scale=1.0 / SQRT_K,
                accum_out=dparts[:, i, nchunk : nchunk + 1],
            )
            t0 += w
            nchunk += 1
        nc.vector.tensor_copy(dsum[:, i : i + 1], dparts[:, i, 0:1])
        for c in range(1, nchunk):
            nc.vector.tensor_add(
                dsum[:, i : i + 1], dsum[:, i : i + 1], dparts[:, i, c : c + 1]
            )
        nc.vector.reciprocal(dinv[:, i : i + 1], dsum[:, i : i + 1])
        if i < NI8:
            # et8 = et * (ESC / rowsum): scale into fp8, zero the masked
            # prefix [0, tstart) explicitly (et8 is whole-row in phase E)
            nc.vector.tensor_scalar_mul(
                dinv[:, i : i + 1], dinv[:, i : i + 1], ESC
            )
            nc.vector.tensor_mul(
                et8[:, i, tstart:],
                et[:, i, tstart:],
                dinv[:, i : i + 1].to_broadcast((P, T - tstart)),
            )
        else:
            nc.vector.tensor_mul(
                et[:, i, tstart:],
                et[:, i, tstart:],
                dinv[:, i : i + 1].to_broadcast((P, T - tstart)),
            )

    # ---------------- phase D: v_own = xst.T @ wv + bv ----------------
    vop = ctx.enter_context(tc.tile_pool(name="vop", bufs=1))
    v8 = vop.tile([P, NI8, VD], F8)
    vbf = vop.tile([P, NI - NI8, VD], CD)
    for jl in range(NI):
        pss = [
            psum.tile([P, SB], F32, tag="ps", name=f"psD{jl}_{vb}")
            for vb in range(VD // SB)
        ]
        if jl < NI8:
            for ci in range(0, NCO, 2):
                for vb in range(VD // SB):
                    nc.tensor.matmul(
                        pss[vb][:],
                        xst8_sb[:, ci : ci + 2, jl * P : (jl + 1) * P],
                        wv8_sb[:, ci : ci + 2, vb * SB : (vb + 1) * SB],
                        start=(ci == 0),
                        stop=(ci == NCO - 2),
                        perf_mode=DR,
                    )
            for vb in range(VD // SB):
                # v8 = (psum + bv)/ESC  ==  psum*(1/ESC) + bv/ESC
                nc.vector.scalar_tensor_tensor(
                    v8[:, jl, vb * SB : (vb + 1) * SB],
                    pss[vb][:],
                    1.0 / ESC,
                    bv16_sb[:, vb * SB : (vb + 1) * SB],
                    ALU.mult,
                    ALU.add,
                )
        else:
            for ci in range(NCO):
                for vb in range(VD // SB):
                    nc.tensor.matmul(
                        pss[vb][:],
                        xstb_sb[:, ci, jl * P : (jl + 1) * P],
                        wv_sb[:, ci, vb * SB : (vb + 1) * SB],
                        start=(ci == 0),
                        stop=(ci == NCO - 1),
                    )
            for vb in range(VD // SB):
                nc.vector.tensor_add(
                    vbf[:, jl - NI8, vb * SB : (vb + 1) * SB],
                    pss[vb][:],
                    bv_sb[:, vb * SB : (vb + 1) * SB],
                )

    # ------------- phase E: read_partial = sum_i et_i.T @ v_i -------------
    read_out = io["read_out"].ap()
    with tc.tile_pool(name="rout", bufs=8) as rout:
        for g in range(T // P):
            ni = g // 2 + 1
            n8 = min(ni, NI8)
            pss = [
                psum.tile([P, SB], F32, tag="ps", name=f"psE{g}_{vb}")
                for vb in range(VD // SB)
            ]
            # ordered (start..stop) op list per vb: fp8 pairs, fp8 single,
            # bf16 tail
            nops = n8 // 2 + n8 % 2 + (ni - n8)
            for vb in range(VD // SB):
                op = 0
                for i in range(0, n8 - 1, 2):
                    nc.tensor.matmul(
                        pss[vb][:],
                        et8[:, i : i + 2, g * P : (g + 1) * P],
                        v8[:, i : i + 2, vb * SB : (vb + 1) * SB],
                        start=(op == 0),
                        stop=(op == nops - 1),
                        perf_mode=DR,
                    )
                    op += 1
                if n8 % 2:
                    i = n8 - 1
                    nc.tensor.matmul(
                        pss[vb][:],
                        et8[:, i, g * P : (g + 1) * P],
                        v8[:, i, vb * SB : (vb + 1) * SB],
                        start=(op == 0),
                        stop=(op == nops - 1),
                    )
                    op += 1
                for i in range(NI8, ni):
                    nc.tensor.matmul(
                        pss[vb][:],
                        et[:, i, g * P : (g + 1) * P],
                        vbf[:, i - NI8, vb * SB : (vb + 1) * SB],
                        start=(op == 0),
                        stop=(op == nops - 1),
                    )
                    op += 1
            for vb in range(VD // SB):
                ro = rout.tile([P, SB], CD, tag="rout")
                if (2 * g + vb) % 2 == 0:
                    nc.scalar.copy(ro[:], pss[vb][:])
                    nc.sync.dma_start(
                        read_out[g * P : (g + 1) * P, vb * SB : (vb + 1) * SB],
                        ro[:],
                    )
                else:
                    nc.vector.tensor_copy(ro[:], pss[vb][:])
                    nc.gpsimd.dma_start(
                        read_out[g * P : (g + 1) * P, vb * SB : (vb + 1) * SB],
                        ro[:],
                    )


def _build_nc(num_devices=8):
    nc = bacc.Bacc(
        "TRN2", target_bir_lowering=False, debug=False, num_devices=num_devices
    )
    io = {}
    io["xt"] = nc.dram_tensor("xt", [C, T], F8, kind="ExternalInput")
    io["xst8"] = nc.dram_tensor("xst8", [C, SOWN], F8, kind="ExternalInput")
    io["xstb"] = nc.dram_tensor("xstb", [C, SOWN], CD, kind="ExternalInput")
    io["wq"] = nc.dram_tensor("wq", [C, KD], F8, kind="ExternalInput")
    io["wk"] = nc.dram_tensor("wk", [C, KD], F8, kind="ExternalInput")
    io["wv"] = nc.dram_tensor("wv", [C, KD], CD, kind="ExternalInput")
    io["wv8"] = nc.dram_tensor("wv8", [C, KD], F8, kind="ExternalInput")
    io["bk2"] = nc.dram_tensor("bk2", [P, NKO], F32, kind="ExternalInput")
    io["bq2"] = nc.dram_tensor("bq2", [P, NKO], F32, kind="ExternalInput")
    io["bv2"] = nc.dram_tensor("bv2", [P, VD], F32, kind="ExternalInput")
    io["bv216"] = nc.dram_tensor("bv216", [P, VD], F32, kind="ExternalInput")
    io["maskbias"] = nc.dram_tensor(
        "maskbias", [P, 2 * P], F32, kind="ExternalInput"
    )
    io["read_out"] = nc.dram_tensor(
        "read_out", [T, VD], CD, kind="ExternalOutput"
    )
    with tile.TileContext(nc) as tc:
        attn_body(tc, io)
    nc.compile()
    return nc


def _own_cols(h):
    idx = []
    for i in range(NI):
        g = 2 * i + h
        idx.extend(range(g * P, (g + 1) * P))
    return np.array(idx)


def _make_in_maps(x, Wq, bq, Wk, bk, Wv, bv):
    import ml_dtypes

    bf16 = ml_dtypes.bfloat16
    f8 = ml_dtypes.float8_e4m3
    x = np.asarray(x, np.float32)
    Wq8, Wk8, Wv8 = (
        np.ascontiguousarray(np.asarray(w, np.float32).astype(f8))
        for w in (Wq, Wk, Wv)
    )
    Wvb = np.ascontiguousarray(np.asarray(Wv, np.float32).astype(bf16))
    bq, bk, bv = (np.asarray(v, np.float32) for v in (bq, bk, bv))

    sr = np.arange(P)[:, None]
    tcc = np.arange(P)[None, :]
    tri = np.where(tcc >= sr, 0.0, -1e9).astype(np.float32)
    masks = {}
    for h in (0, 1):
        m = np.zeros((P, 2 * P), np.float32)
        if h == 0:
            m[:, :P] = tri
        else:
            m[:, :P] = -1e9
            m[:, P:] = tri
        masks[h] = m

    bk2 = np.ascontiguousarray(bk.reshape(NKO, P).T)
    bq2 = np.ascontiguousarray(bq.reshape(NKO, P).T)
    bv2 = np.ascontiguousarray(np.broadcast_to(bv[None, :], (P, VD)))
    bv216 = np.ascontiguousarray(bv2 / ESC)

    in_maps = []
    for core in range(8):
        b, h = core // 2, core % 2
        xt_b = np.ascontiguousarray(x[b].T)
        own = _own_cols(h)
        in_maps.append(
            {
                "xt": np.ascontiguousarray(xt_b.astype(f8)),
                "xst8": np.ascontiguousarray(xt_b[:, own].astype(f8)),
                "xstb": np.ascontiguousarray(xt_b[:, own].astype(bf16)),
                "wq": Wq8,
                "wk": Wk8,
                "wv": Wvb,
                "wv8": Wv8,
                "bk2": bk2,
                "bq2": bq2,
                "bv2": bv2,
                "bv216": bv216,
                "maskbias": masks[h],
            }
        )
    return in_maps


def _assemble_output(x, results):
    x = np.asarray(x, np.float32)
    out = np.empty((x.shape[0], T, C + VD), np.float32)
    out[:, :, :C] = x
    for b in range(x.shape[0]):
        out[b, :, C:] = results[2 * b]["read_out"].astype(np.float32) + results[
            2 * b + 1
        ]["read_out"].astype(np.float32)
    return out


_NC_CACHE = None


def _build():
    global _NC_CACHE
    if _NC_CACHE is None:
        _NC_CACHE = _build_nc(num_devices=8)
    return _NC_CACHE


def kernel(x, Wq, bq, Wk, bk, Wv, bv):
    from concourse.bass_utils import run_bass_kernel_spmd

    nc = _build()
    in_maps = _make_in_maps(x, Wq, bq, Wk, bk, Wv, bv)
    res = run_bass_kernel_spmd(nc, in_maps, core_ids=list(range(8)))
    return _assemble_output(x, res.results)
